# revision 3
# baseline (speedup 1.0000x reference)
"""kernel4: dst side via chunk-pure dma_gather (1024 idxs/instr, 4 queues).
Edges segmented by dst chunk (dst>>15 -> 4 chunks, passed as 4 separate dram
tensors so int16 idx + 16MB reach suffice). Within each segment, src is
banded per-segment: nodes ordered by descending per-segment src-degree;
windows of KSL consecutive rows of that ordering; slabs stream from one
concatenated tensor zq = concat_c z[sigma_c] via contiguous-run indirect DMA
(runtime int32 bases). Segment window lists padded to 128-window multiples so
every region (128 windows) is segment-pure and every dma_gather instruction
(8 slot-cols = 1024 slots) draws from a single chunk tensor."""

import numpy as np

import concourse.bass as bass
import concourse.mybir as mybir
import concourse.tile as tile
from concourse import bacc
from concourse.bass import IndirectOffsetOnAxis
from concourse.bass_utils import run_bass_kernel_spmd
from concourse.masks import make_identity
from contextlib import ExitStack

N, D, H = 100000, 128, 128
E_TOTAL = 2000000
NCORES = 8
P = 128
KSL = 24
NPAD = 32
E_CORE = E_TOTAL // NCORES
CH = 32768
NCHUNK = 4
GPI = 3            # dma_gather instrs per region (3 x 8 cols = 24)
NI = 1024          # idxs per dma_gather

F32 = mybir.dt.float32
BF16 = mybir.dt.bfloat16
I32 = mybir.dt.int32
I16 = mybir.dt.int16
RELU = mybir.ActivationFunctionType.Relu
IDENT = mybir.ActivationFunctionType.Identity


def build_program(nregs, n=N):
    nreg = sum(nregs)
    nc = bacc.Bacc("TRN2", target_bir_lowering=False, debug=False,
                   enable_asserts=False, num_devices=NCORES,
                   num_swdge_queues=4)
    zq_d = nc.dram_tensor("zq", [NCHUNK * (n + NPAD), D], F32,
                          kind="ExternalInput").ap()
    zd_d = [nc.dram_tensor(f"zd{c}", [min(CH, n - c * CH), D], F32,
                           kind="ExternalInput").ap() for c in range(NCHUNK)]
    sb_d = nc.dram_tensor("sb", [P, nreg], I32, kind="ExternalInput").ap()
    di_d = nc.dram_tensor("di", [P, nreg * GPI * (NI // 16)], I16,
                          kind="ExternalInput").ap()
    w1_d = nc.dram_tensor("w1", [D, H], F32, kind="ExternalInput").ap()
    b1_d = nc.dram_tensor("b1", [H], F32, kind="ExternalInput").ap()
    w2_d = nc.dram_tensor("w2", [H, 1], F32, kind="ExternalInput").ap()
    b2_d = nc.dram_tensor("b2r", [P], F32, kind="ExternalInput").ap()
    out_d = nc.dram_tensor("out", [nreg * KSL * P], F32,
                           kind="ExternalOutput").ap()

    with tile.TileContext(nc) as tc, ExitStack() as ctx:
        const = ctx.enter_context(tc.tile_pool(name="const", bufs=1))
        zz = ctx.enter_context(tc.tile_pool(name="slabp", bufs=4))
        dzp = ctx.enter_context(tc.tile_pool(name="dstp", bufs=4))
        work = ctx.enter_context(tc.tile_pool(name="work", bufs=4))
        stage_pool = ctx.enter_context(tc.tile_pool(name="stage", bufs=2))
        ps_t = ctx.enter_context(tc.tile_pool(name="ps_t", bufs=3, space="PSUM"))
        ps_h = ctx.enter_context(tc.tile_pool(name="ps_h", bufs=2, space="PSUM"))
        ps_o = ctx.enter_context(tc.tile_pool(name="ps_o", bufs=2, space="PSUM"))

        sb_sb = const.tile([P, nreg], I32)
        nc.sync.dma_start(out=sb_sb[:], in_=sb_d[:, :])
        CI = NI // 16
        di_sb = const.tile([P, nreg * GPI * CI], I16)
        nc.sync.dma_start(out=di_sb[:], in_=di_d[:, :])
        w1_sb = const.tile([P, H], F32)
        nc.sync.dma_start(out=w1_sb[:], in_=w1_d[:, :])
        b1_sb = const.tile([P, 1], F32)
        nc.sync.dma_start(out=b1_sb[:], in_=b1_d[:, None])
        w2_sb = const.tile([P, 1], F32)
        nc.sync.dma_start(out=w2_sb[:], in_=w2_d[:, :])
        b2_sb = const.tile([P, 1], F32)
        nc.sync.dma_start(out=b2_sb[:], in_=b2_d[:, None])
        ident = const.tile([P, P], F32)
        make_identity(nc, ident[:])
        w1_bf = const.tile([P, H], BF16)
        nc.vector.tensor_copy(out=w1_bf[:], in_=w1_sb[:])
        w2_bf = const.tile([P, 1], BF16)
        nc.vector.tensor_copy(out=w2_bf[:], in_=w2_sb[:])
        ident_bf = const.tile([P, P], BF16)
        nc.vector.tensor_copy(out=ident_bf[:], in_=ident[:])

        ob = 0
        gq = 0

        def block(slab, dz, b, o_ps):
            ef = work.tile([P, 512], BF16, tag="ef")
            nc.vector.tensor_mul(out=ef[:],
                                 in0=slab[:, 4 * b * D : (4 * b + 4) * D],
                                 in1=dz[:, 4 * b * D : (4 * b + 4) * D])
            efT_ps = ps_t.tile([P, 512], BF16)
            for j in range(4):
                nc.tensor.transpose(
                    out=efT_ps[:, j * P : (j + 1) * P],
                    in_=ef[:, j * P : (j + 1) * P],
                    identity=ident_bf[:])
            efT = work.tile([P, 512], BF16, tag="efT")
            nc.scalar.activation(out=efT[:], in_=efT_ps[:], func=IDENT,
                                 bias=0.0, scale=1.0)
            h_ps = ps_h.tile([P, 512], F32)
            nc.tensor.matmul(out=h_ps[:], lhsT=w1_bf[:], rhs=efT[:],
                             start=True, stop=True)
            h_sb = work.tile([P, 512], BF16, tag="h")
            nc.scalar.activation(out=h_sb[:], in_=h_ps[:], func=RELU,
                                 bias=b1_sb[:, :1], scale=1.0)
            for c in range(4):
                nc.tensor.matmul(out=o_ps[:, 4 * b + c : 4 * b + c + 1],
                                 lhsT=h_sb[:, c * P : (c + 1) * P],
                                 rhs=w2_bf[:], start=True, stop=True)

        ri = 0
        for c in range(NCHUNK):
            for _ in range(nregs[c]):
                slab = zz.tile([P, KSL * D], F32, tag="slab")
                nc.gpsimd.indirect_dma_start(
                    out=slab[:], out_offset=None, in_=zq_d[:, :],
                    in_offset=IndirectOffsetOnAxis(
                        ap=sb_sb[:, ri : ri + 1], axis=0))
                dz = dzp.tile([P, KSL * D], F32, tag="dst")
                for k in range(GPI):
                    nc.gpsimd.dma_gather(
                        out_ap=dz[:, k * 8 * D : (k + 1) * 8 * D].rearrange(
                            "p (k e) -> p k e", e=D),
                        in_ap=zd_d[c][:, :],
                        idxs_ap=di_sb[:, (ri * GPI + k) * CI :
                                      (ri * GPI + k + 1) * CI],
                        num_idxs=NI, num_idxs_reg=NI, elem_size=D,
                        queue_num=gq % 4)
                    gq += 1
                o_ps = ps_o.tile([P, KSL], F32)
                for b in range(KSL // 4):
                    block(slab, dz, b, o_ps)
                oT = stage_pool.tile([P, KSL], F32, tag="oT")
                nc.scalar.activation(out=oT[:], in_=o_ps[:], func=IDENT,
                                     bias=b2_sb[:, :1], scale=1.0)
                nc.sync.dma_start(
                    out=out_d[ob : ob + P * KSL].rearrange("(p j) -> p j",
                                                           p=P),
                    in_=oT[:])
                ob += P * KSL
                ri += 1

    nc.compile()
    return nc


def _windows_for(subS, subE, segbase):
    """Windows for one segment: subS = src ids of segment edges, subE = their
    original edge ids. Returns (order_nodes, sbase_local, dstslot_eids
    [nw, KSL] int64 edge ids or -1)."""
    deg = np.bincount(subS, minlength=N)
    order_nodes = np.argsort(-deg, kind="stable")
    rank = np.empty(N, np.int64)
    rank[order_nodes] = np.arange(N)
    eorder = np.argsort(rank[subS], kind="stable")
    sdeg = deg[order_nodes]
    na = int((sdeg > 0).sum())
    seg = np.zeros(na + 1, np.int64)
    np.cumsum(sdeg[:na], out=seg[1:])
    dmax = int(sdeg[0]) if na else 0
    wins = []
    for r in range(1, dmax + 1):
        c_r = int((sdeg[:na] >= r).sum())
        for j in range(-(-c_r // KSL)):
            wins.append((r, j * KSL))
    nw = len(wins)
    sbase = np.zeros(nw, np.int32)
    eids = np.full((nw, KSL), -1, np.int64)
    for w, (r, base) in enumerate(wins):
        sbase[w] = base
        rows = np.arange(base, base + KSL)
        valid = rows < na
        vr = rows[valid]
        has = sdeg[vr] >= r
        vr = vr[has]
        sl = np.where(valid)[0][has]
        eids[w, sl] = subE[eorder[seg[vr] + (r - 1)]]
    return order_nodes, sbase, eids


def pack_core(S, Dd):
    """Returns (zq_orders [4][N], sbase_g [nw_tot], eids [nw_tot, KSL],
    nregs [4])."""
    ch = (Dd // CH).astype(np.int64)
    allE = np.arange(len(S), dtype=np.int64)
    orders, sb_list, eid_list, nregs = [], [], [], []
    for c in range(NCHUNK):
        m = ch == c
        order_nodes, sbase, eids = _windows_for(S[m], allE[m], c)
        nw = len(sbase)
        nwp = -(-nw // P) * P if nw else P
        sb = np.zeros(nwp, np.int32)
        ei = np.full((nwp, KSL), -1, np.int64)
        sb[:nw] = sbase + c * (N + NPAD)     # global zq row base
        sb[nw:] = c * (N + NPAD)
        ei[:nw] = eids
        orders.append(order_nodes)
        sb_list.append(sb)
        eid_list.append(ei)
        nregs.append(nwp // P)
    return (orders, np.concatenate(sb_list),
            np.concatenate(eid_list, axis=0), nregs)


def pack_all(edge_label_index):
    src_f = np.asarray(edge_label_index[0], dtype=np.int64)
    dst_f = np.asarray(edge_label_index[1], dtype=np.int64)
    cores = []
    for c in range(NCORES):
        sl = slice(c * E_CORE, (c + 1) * E_CORE)
        cores.append((pack_core(src_f[sl], dst_f[sl]), dst_f[sl]))
    # unify nregs across cores (pad each segment to the max)
    nregs = [max(x[0][3][c] for x in cores) for c in range(NCHUNK)]
    packed = []
    for (orders, sb, ei, nr), dsub in cores:
        sbs, eis = [], []
        off = 0
        for c in range(NCHUNK):
            nwp = nr[c] * P
            want = nregs[c] * P
            s = np.full(want, c * (N + NPAD), np.int32)
            e = np.full((want, KSL), -1, np.int64)
            s[:nwp] = sb[off : off + nwp]
            e[:nwp] = ei[off : off + nwp]
            off += nwp
            sbs.append(s)
            eis.append(e)
        sb_g = np.concatenate(sbs)
        ei_g = np.concatenate(eis, axis=0)
        nreg = sum(nregs)
        # window w -> region w//P, lane w%P
        sbT = sb_g.reshape(nreg, P).T.copy()             # [P, nreg]
        org = ei_g.reshape(nreg, P, KSL)                 # [nreg][p][s]
        # dst idx (chunk-local) per slot; dummies -> 0
        dloc = np.zeros((nreg, P, KSL), np.int64)
        valid = org >= 0
        dloc[valid] = dsub[org[valid]] % CH
        # build int16 wrapped idx lists: per region, per instr k (8 cols),
        # flat j = col*128 + p  (col within the 8-col group)
        di16 = np.zeros((P, nreg * GPI * (NI // 16)), np.int16)
        for i in range(nreg):
            for k in range(GPI):
                grid = dloc[i, :, k * 8 : (k + 1) * 8]   # [P, 8]
                flat = grid.T.ravel().astype(np.int16)   # [1024]
                blk = flat.reshape(NI // 16, 16).T       # [16, NI/16]
                c0 = (i * GPI + k) * (NI // 16)
                di16[:, c0 : c0 + NI // 16] = np.tile(blk, (8, 1))
        packed.append((orders, np.ascontiguousarray(sbT),
                       np.ascontiguousarray(di16), org))
    return packed, tuple(nregs)


_NC_CACHE = {}


def run(inputs, trace=False, **kw):
    z = np.ascontiguousarray(np.asarray(inputs["z"], dtype=np.float32))
    w1 = np.ascontiguousarray(np.asarray(inputs["W1"], dtype=np.float32))
    b1v = np.ascontiguousarray(np.asarray(inputs["b1"], dtype=np.float32))
    w2 = np.ascontiguousarray(np.asarray(inputs["W2"], dtype=np.float32))
    b2v = np.ascontiguousarray(np.asarray(inputs["b2"], dtype=np.float32))
    b2r = np.ascontiguousarray(np.broadcast_to(b2v.reshape(1), (P,)).copy())
    packed, nregs = pack_all(inputs["edge_label_index"])
    if nregs not in _NC_CACHE:
        _NC_CACHE[nregs] = build_program(nregs)
    pad = np.zeros((NPAD, D), np.float32)
    zds = {f"zd{c}": np.ascontiguousarray(z[c * CH : min((c + 1) * CH, N)])
           for c in range(NCHUNK)}
    in_maps = []
    for orders, sbT, di16, _ in packed:
        zq = np.ascontiguousarray(np.concatenate(
            [np.concatenate([z[o], pad], axis=0) for o in orders], axis=0))
        m = {"zq": zq, "sb": sbT, "di": di16, "w1": w1, "b1": b1v,
             "w2": w2, "b2r": b2r}
        m.update(zds)
        in_maps.append(m)
    res = run_bass_kernel_spmd(_NC_CACHE[nregs], in_maps,
                               list(range(NCORES)), trace=trace, **kw)
    outs = []
    nreg = sum(nregs)
    for c in range(NCORES):
        dev = res.results[c]["out"].reshape(nreg, P, KSL)
        org = packed[c][3]
        valid = org >= 0
        full = np.zeros(E_CORE, np.float32)
        full[org[valid]] = dev[valid]
        outs.append(full)
    return np.concatenate(outs).astype(np.float32), res


def kernel(z, edge_label_index, W1, b1, W2, b2):
    out, _ = run({"z": z, "edge_label_index": edge_label_index,
                  "W1": W1, "b1": b1, "W2": W2, "b2": b2})
    return out
